# revision 53
# baseline (speedup 1.0000x reference)
"""Dinov3 self-attention Bass kernel for TRN2.

Sharding: data-parallel over batch. B=8 batch elements -> 8 NeuronCores,
one full attention per core, weights replicated. No collectives.

v2 design (HAM-aware): keep the PE stream dense so the HAM clock gate
stays at 8/8 (2.4 GHz).  Per head-pair pt:
  - q/k projections land transposed (qT/kT [d, s]), RoPE'd in place.
  - v lands natural in vsb [j, (12 heads x 64 | 64 ones)]; the 64 ones
    columns make every PV matmul also produce the softmax denominator
    REPLICATED on psum rows 64..127 (M=128 costs the same as M=65).
  - attention per (pt, ic, jt): one [jsz, 1024] psum holds BOTH heads'
    score chunks (h0 cols 0:512, h1 cols 512:1024) -> ONE wide exp on
    the scalar engine -> two PV matmuls accumulate ctx+denom.
  - normalize fully on-chip: DVE reciprocal_approx_fast on the
    replicated denominator rows + DVE multiply -> ctxT bf16.  No DRAM
    bounce, no 4-byte scatter DMAs.
  - projections for pt+1 are interleaved between the QK and PV groups
    of pt's attention so the PE never idles while exp runs; for pt=5
    the first output-projection tiles are the filler.
Engines: PE matmuls; ACT exp (+ tail out-proj evicts); DVE everything
elementwise; GpSimd queue posts the xbar transposes; Sync queue posts
loads/stores.
"""

import contextlib
import sys

import numpy as np

sys.path.insert(0, "/opt/trn_rl_repo")

import concourse.bacc as bacc
import concourse.bass as bass
import concourse.tile as tile
from concourse import mybir

S = 1374
H = 768
NH = 12
D = 64
NROT = 1369
PREFIX = S - NROT  # 5
B = 8

P = 128
NSTILE = (S + P - 1) // P  # 11 s-tiles, last has 94 rows
NOTILE = H // P  # 6 head pairs
SPAD = NSTILE * P  # 1408
IC = ((0, 512), (512, 512), (1024, 350))  # i-chunks, each <= 1 psum bank

F32 = mybir.dt.float32
BF16 = mybir.dt.bfloat16


def _stile(i):
    start = i * P
    return start, min(P, S - start)


def build_kernel(nc):
    x_ext = nc.declare_dram_parameter("hidden_states", [S, H], F32, isOutput=False)
    sin_ext = nc.declare_dram_parameter("sin", [NROT, D], F32, isOutput=False)
    cos_ext = nc.declare_dram_parameter("cos", [NROT, D], F32, isOutput=False)
    wq_ext = nc.declare_dram_parameter("Wq", [H, H], F32, isOutput=False)
    bq_ext = nc.declare_dram_parameter("bq", [H], F32, isOutput=False)
    wk_ext = nc.declare_dram_parameter("Wk", [H, H], F32, isOutput=False)
    wv_ext = nc.declare_dram_parameter("Wv", [H, H], F32, isOutput=False)
    bv_ext = nc.declare_dram_parameter("bv", [H], F32, isOutput=False)
    wp_ext = nc.declare_dram_parameter("Wp", [H, H], F32, isOutput=False)
    bp_ext = nc.declare_dram_parameter("bp", [H], F32, isOutput=False)
    out_ext = nc.declare_dram_parameter("out", [S, H], F32, isOutput=True)

    with tile.TileContext(nc) as tc:
        _body(tc, x_ext, sin_ext, cos_ext, wq_ext, bq_ext, wk_ext,
              wv_ext, bv_ext, wp_ext, bp_ext, out_ext)
    nc.compile()
    return nc


def _body(tc, x_ext, sin_ext, cos_ext, wq_ext, bq_ext, wk_ext, wv_ext,
          bv_ext, wp_ext, bp_ext, out_ext):
    nc = tc.nc

    with contextlib.ExitStack() as ctx:
        persist = ctx.enter_context(tc.tile_pool(name="persist", bufs=1))
        psum_sc = ctx.enter_context(tc.tile_pool(name="psum_sc", bufs=2, space="PSUM"))
        psum_pv = ctx.enter_context(tc.tile_pool(name="psum_pv", bufs=1, space="PSUM"))
        psum_pj = ctx.enter_context(tc.tile_pool(name="psum_pj", bufs=2, space="PSUM"))
        es_pool = ctx.enter_context(tc.tile_pool(name="es_pool", bufs=4))
        rec_pool = ctx.enter_context(tc.tile_pool(name="rec_pool", bufs=2))
        ropet = ctx.enter_context(tc.tile_pool(name="ropet", bufs=2))
        stage = ctx.enter_context(tc.tile_pool(name="stage", bufs=2))
        cstage = ctx.enter_context(tc.tile_pool(name="cstage", bufs=1))
        outst = ctx.enter_context(tc.tile_pool(name="outst", bufs=2))

        xT = persist.tile([P, NOTILE, SPAD], BF16)     # xT[p, t, s] = x[s, 128t+p]
        qT = persist.tile([P, NOTILE, SPAD], BF16)     # roped q, [(hh,d), pt, s]
        kT = persist.tile([P, NOTILE, SPAD], BF16)
        ctxT = persist.tile([P, NOTILE, SPAD], BF16)   # normalized ctx^T
        # per head: 64 ones columns + 64 v columns, so every PV matmul
        # (lhsT = one head slot, M=128) also produces the denominator
        # replicated on psum rows 0-63 (ctx on rows 64-127).  The ones
        # block comes FIRST so the reciprocal reads psum at partition
        # offset 0 — HW lowering drops nonzero partition offsets on the
        # custom-DVE reciprocal input.
        vsb = persist.tile([P, NSTILE, NH, 2 * D], BF16)
        wqT = persist.tile([P, NOTILE, H], BF16)
        wkT = persist.tile([P, NOTILE, H], BF16)
        wvT = persist.tile([P, NOTILE, H], BF16)
        wpT = persist.tile([P, NOTILE, H], BF16)
        cc2 = persist.tile([P, SPAD], BF16)            # cos^T stacked twice
        ss2 = persist.tile([P, SPAD], BF16)            # sin^T stacked, sign-baked
        bq_sb = persist.tile([P, NOTILE], F32)
        bv_row = persist.tile([1, H], BF16)
        bp_row = persist.tile([1, H], BF16)
        ones_row = persist.tile([1, P], BF16)
        ident = persist.tile([P, P], BF16)

        nc.vector.memset(ones_row, 1.0)
        from concourse.masks import make_identity
        make_identity(nc, ident)

        # ---------------- load helpers ----------------
        def load_biases():
            nc.sync.dma_start(out=bq_sb, in_=bq_ext.rearrange("(t p) -> p t", p=P))
            # 1-lane casts are ~6x faster on ACT than DVE, and the DVE
            # queue feeds the startup-critical x casts
            bstage = stage.tile([1, H], F32, tag="bias_stage", bufs=1)
            nc.sync.dma_start(out=bstage, in_=bv_ext.rearrange("(a h) -> a h", a=1))
            nc.scalar.copy(out=bv_row, in_=bstage)
            bstage2 = stage.tile([1, H], F32, tag="bias_stage2", bufs=1)
            nc.sync.dma_start(out=bstage2, in_=bp_ext.rearrange("(a h) -> a h", a=1))
            nc.scalar.copy(out=bp_row, in_=bstage2)

        def load_sincos():
            # [NROT, 64] f32 -> bf16 [64, s] via one batched xbar transpose,
            # then copy into both halves of [128, s]; bake rotate_half signs.
            n_rtile = (NROT + P - 1) // P
            for src_ext, dstT in ((cos_ext, cc2), (sin_ext, ss2)):
                cst_all = cstage.tile([P, SPAD], F32, tag="cs_stage")
                csb_all = cstage.tile([P, SPAD], BF16, tag="cs_stage_bf")
                csT3 = cstage.tile([P, n_rtile, P], BF16, tag="cs_T3")
                nc.gpsimd.memset(csb_all, 0.0)
                for i in range(n_rtile):
                    r0 = i * P
                    rsz = min(P, NROT - r0)
                    nc.sync.dma_start(out=cst_all[:rsz, i * P:i * P + D],
                                      in_=src_ext[r0:r0 + rsz, :])
                    nc.vector.tensor_copy(
                        out=csb_all[:rsz, i * P:i * P + D],
                        in_=cst_all[:rsz, i * P:i * P + D])
                nc.scalar.dma_start_transpose(out=csT3, in_=csb_all)
                # scatter copies on ACT: the scalar queue is empty during
                # the prologue while DVE feeds the startup-critical x casts
                for i in range(n_rtile):
                    r0 = i * P
                    rsz = min(P, NROT - r0)
                    for half in range(2):
                        nc.scalar.copy(
                            out=dstT[64 * half:64 * half + 64, r0:r0 + rsz],
                            in_=csT3[0:D, i, :rsz])
            for base in (0, 64):
                sl = slice(base, base + 32)
                nc.vector.tensor_scalar_mul(ss2[sl, :NROT], ss2[sl, :NROT], -1.0)

        _xevict = []

        def load_x_tile(st):
            # transpose on the PE (idle at startup) — the xbar path costs
            # a 1.25us scalar-queue post per tile plus sem-chained stage
            # slots, which serialized the whole startup at ~6us/tile.
            # The psum evict is deferred one tile so it never head-of-line
            # blocks the next tile's cast on the DVE FIFO.
            s0, ssz = _stile(st)
            xs = stage.tile([P, H], F32, tag="x_stage", name=f"xs_{st}")
            xb = stage.tile([P, H], BF16, tag="x_stage_bf", name=f"xb_{st}")
            if ssz < P:
                nc.vector.memset(xb, 0.0)
            nc.sync.dma_start(out=xs[:ssz], in_=x_ext[s0:s0 + ssz, :])
            nc.vector.tensor_copy(out=xb[:ssz], in_=xs[:ssz])
            xtp = psum_sc.tile([P, H], BF16, tag="sc", name=f"xtp_{st}")
            for kt in range(NOTILE):
                nc.tensor.transpose(xtp[:, kt * P:(kt + 1) * P],
                                    xb[:, kt * P:(kt + 1) * P], ident)
            _xevict.append((xtp, s0))
            if len(_xevict) > 1:
                flush_xevict(1)

        def flush_xevict(keep=0):
            while len(_xevict) > keep:
                xtp, s0 = _xevict.pop(0)
                nc.vector.tensor_copy(
                    out=xT[:, :, s0:s0 + P],
                    in_=xtp.rearrange("p (k q) -> p k q", q=P))

        def load_w_row(w_ext, wT, r, tq=None):
            # tq: engine queue for the xbar-transpose post.  The scalar
            # queue feeds the exps, so only startup-critical transposes
            # go there; the rest post from the (idle) sync queue.
            ws = stage.tile([P, H], F32, tag="w_stage", name=f"ws_{wT.name}_{r}")
            wb = stage.tile([P, H], BF16, tag="w_stage_bf", name=f"wb_{wT.name}_{r}")
            nc.sync.dma_start(out=ws, in_=w_ext[r * P:(r + 1) * P, :])
            nc.vector.tensor_copy(out=wb, in_=ws)
            (tq or nc.scalar).dma_start_transpose(
                out=wT[:, :, r * P:(r + 1) * P], in_=wb)

        # ---------------- projection emit-units ----------------
        _pj_live = {}

        def qkproj_half(wT, dst, ot, ci, bias, half, act_evict=False):
            # half 0: kts 0-2 (allocates psum); half 1: kts 3-5 + evict.
            # Split so paced filling can interleave at ~0.6us granularity.
            # act_evict: in the prologue ACT is idle and the DVE FIFO feeds
            # the startup-critical x casts — evict there instead.
            i0, ilen = IC[ci]
            key = (wT.name, ot, ci)
            if half == 0:
                _pj_live[key] = psum_pj.tile(
                    [P, 512], F32, tag="pj",
                    name=f"qk_{dst.name}_{ot}_{ci}")[:, :ilen]
            pj = _pj_live[key]
            for kt in range(3 * half, 3 * half + 3):
                nc.tensor.matmul(
                    pj, wT[:, kt, ot * P:(ot + 1) * P],
                    xT[:, kt, i0:i0 + ilen],
                    start=(kt == 0), stop=(kt == NOTILE - 1))
            if half == 1:
                del _pj_live[key]
                if bias:
                    if act_evict:
                        nc.scalar.add(dst[:, ot, i0:i0 + ilen], pj,
                                      bq_sb[:, ot:ot + 1])
                    else:
                        nc.vector.tensor_scalar_add(dst[:, ot, i0:i0 + ilen],
                                                    pj, bq_sb[:, ot:ot + 1])
                elif act_evict:
                    nc.scalar.copy(out=dst[:, ot, i0:i0 + ilen], in_=pj)
                else:
                    nc.vector.tensor_copy(out=dst[:, ot, i0:i0 + ilen], in_=pj)

        _rope_live = {}

        def rope_dma(dst, ot):
            rot = ropet.tile([P, NROT], BF16, tag="rot", name=f"rot_{dst.name}_{ot}")
            _rope_live[(dst.name, ot)] = rot
            sl = slice(PREFIX, PREFIX + NROT)
            for (dst0, src0) in ((0, 32), (32, 0), (64, 96), (96, 64)):
                nc.sync.dma_start(
                    out=rot[dst0:dst0 + 32, :],
                    in_=dst[src0:src0 + 32, ot, sl])

        def rope_mul(dst, ot):
            # separate unit: the in-place mul WAR-waits on the rotate DMAs;
            # emitting it later keeps that wait off the DVE FIFO head
            rot = _rope_live.pop((dst.name, ot))
            sl = slice(PREFIX, PREFIX + NROT)
            nc.vector.tensor_mul(dst[:, ot, sl], dst[:, ot, sl], cc2[:, :NROT])
            nc.vector.tensor_mul(rot, rot, ss2[:, :NROT])
            nc.vector.tensor_add(dst[:, ot, sl], dst[:, ot, sl], rot)

        def rope(dst, ot):
            rope_dma(dst, ot)
            rope_mul(dst, ot)

        def vproj_st(pt, st):
            s0, ssz = _stile(st)
            pj = psum_pj.tile([P, 512], F32, tag="pj",
                              name=f"v_{pt}_{st}")[:, :P]
            for kt in range(NOTILE):
                nc.tensor.matmul(
                    pj[:ssz, :], xT[:, kt, s0:s0 + ssz],
                    wvT[:, kt, pt * P:(pt + 1) * P],
                    start=(kt == 0), stop=False)
            nc.tensor.matmul(
                pj[:ssz, :], ones_row[:, :ssz],
                bv_row[:, pt * P:(pt + 1) * P],
                start=False, stop=True)
            nc.vector.tensor_copy(
                out=vsb[:ssz, st, 2 * pt:2 * pt + 2, D:2 * D],
                in_=pj[:ssz, :].rearrange("p (h d) -> p h d", d=D))

        def outproj_it(it):
            s0, ssz = _stile(it)
            ot_t = outst.tile([P, H], F32, tag="ostage", name=f"ost_{it}")
            for ci, (o0, on) in enumerate(((0, 512), (512, 256))):
                pj = psum_pj.tile([P, 512], F32, tag="pj",
                                  name=f"o_{it}_{ci}")[:, :on]
                for kt in range(NOTILE):
                    nc.tensor.matmul(
                        pj[:ssz, :], ctxT[:, kt, s0:s0 + ssz],
                        wpT[:, kt, o0:o0 + on],
                        start=(kt == 0), stop=False)
                nc.tensor.matmul(
                    pj[:ssz, :], ones_row[:, :ssz], bp_row[:, o0:o0 + on],
                    start=False, stop=True)
                nc.scalar.copy(out=ot_t[:ssz, o0:o0 + on], in_=pj[:ssz, :])
            nc.sync.dma_start(out=out_ext[s0:s0 + ssz, :], in_=ot_t[:ssz])

        # ---------------- emission order ----------------
        # wave 1: exactly what the first projection matmuls need (x tiles
        # 0-3 + row 0 of Wq/Wk/Wv), projection chunks interleaved with the
        # remaining x tiles so the PE queue pipelines through the startup.
        for st in range(4):
            load_x_tile(st)
        load_w_row(wq_ext, wqT, 0)
        load_w_row(wk_ext, wkT, 0)
        load_w_row(wv_ext, wvT, 0)
        load_biases()
        flush_xevict()
        qkproj_half(wqT, qT, 0, 0, True, 0, act_evict=True)
        qkproj_half(wqT, qT, 0, 0, True, 1, act_evict=True)
        for st in range(4, 8):
            load_x_tile(st)
        flush_xevict()
        qkproj_half(wqT, qT, 0, 1, True, 0, act_evict=True)
        qkproj_half(wqT, qT, 0, 1, True, 1, act_evict=True)
        for st in range(8, NSTILE):
            load_x_tile(st)
        flush_xevict()
        qkproj_half(wqT, qT, 0, 2, True, 0, act_evict=True)
        qkproj_half(wqT, qT, 0, 2, True, 1, act_evict=True)
        # sincos after all x tiles: its 22 staging DMAs would otherwise
        # block the x loads on the sync queue (cos->sin stage-tile WAR)
        load_sincos()
        nc.gpsimd.memset(vsb[:, :, :, 0:D], 1.0)
        for ci in range(3):
            qkproj_half(wkT, kT, 0, ci, False, 0, act_evict=True)
            qkproj_half(wkT, kT, 0, ci, False, 1, act_evict=True)
        rope(qT, 0)
        rope(kT, 0)
        for st in range(NSTILE):
            vproj_st(0, st)

        # row 1 of each weight feeds proj(1), the attention(0) filler
        load_w_row(wq_ext, wqT, 1, tq=nc.sync)
        load_w_row(wk_ext, wkT, 1, tq=nc.sync)
        load_w_row(wv_ext, wvT, 1, tq=nc.sync)

        def vhead_ap(jsz, jt, h):
            return vsb[:jsz, jt, h, :]

        def two_run_ap(t, rows, ilen):
            """[rows, 2, ilen] AP over a [P, 1024] tile: cols {0:ilen} and
            {512:512+ilen} — skips the unwritten hole when ilen < 512.
            For full-width chunks a flat 2D AP is equivalent and cheaper."""
            s = t[:rows, :]
            if ilen == 512:
                return s
            dims = [list(d) for d in s.ap]
            st = dims[-1][0]
            return bass.AP(tensor=s.tensor, offset=s.offset,
                           ap=[dims[0], [512 * st, 2], [st, ilen]])

        exp_f = mybir.ActivationFunctionType.Exp
        scaling = float(D) ** -0.5
        flush_norm = [lambda: None]

        for pt in range(NOTILE):
            # filler units: projections for pt+1 (for pt=4: only the first
            # 3 v-proj tiles — the rest fill attention(5, ic0) itself),
            # Wp loads during attention(0), out-proj row-tiles for pt=5.
            # just-in-time weight streaming: row pt+2 of Wq/Wk/Wv (feeds
            # proj(pt+2)) and one Wp row per pt — spread so no queue ever
            # sees a burst of weight traffic.
            fills = []
            if pt + 2 < NOTILE:
                for w_ext, wT in ((wq_ext, wqT), (wk_ext, wkT), (wv_ext, wvT)):
                    fills.append(lambda w_ext=w_ext, wT=wT:
                                 load_w_row(w_ext, wT, pt + 2, tq=nc.sync))
            if pt < NOTILE - 1:
                fills.append(lambda pt=pt: load_w_row(wp_ext, wpT, pt, tq=nc.sync))
                if pt == NOTILE - 2:
                    fills.append(lambda: load_w_row(wp_ext, wpT, NOTILE - 1,
                                                    tq=nc.sync))
            if pt + 1 < NOTILE:
                np1 = pt + 1
                for ci3 in range(3):
                    for half in range(2):
                        fills.append(lambda ci3=ci3, half=half, np1=np1:
                                     qkproj_half(wqT, qT, np1, ci3, True, half))
                fills.append(lambda np1=np1: rope_dma(qT, np1))
                for ci3 in range(3):
                    for half in range(2):
                        fills.append(lambda ci3=ci3, half=half, np1=np1:
                                     qkproj_half(wkT, kT, np1, ci3, False, half))
                fills.append(lambda np1=np1: rope_dma(kT, np1))
                fills.append(lambda np1=np1: rope_mul(qT, np1))
                fills.append(lambda np1=np1: rope_mul(kT, np1))
                vmax = NSTILE if np1 < NOTILE - 1 else 3
                for st in range(vmax):
                    fills.append(lambda st=st, np1=np1: vproj_st(np1, st))
            # (for pt=5 the rest of v-proj(5) is emitted inline in the ic0
            # jt loop below — emission order must stay ahead of the PV
            # reads, since Tile tracks dependencies in trace order.)
            stage_fills = {}
            if pt == NOTILE - 1:
                # it 0-3 need ctxT i cols 0:512 (ready after ic0's
                # normalize); it 4-7 need cols up to 1024 (after ic1).
                stage_fills[1] = [lambda it=it: outproj_it(it) for it in range(4)]
                stage_fills[2] = [lambda it=it: outproj_it(it) for it in range(4, 8)]

            state = [0, 0]  # units emitted, paces done (of 39)

            def pace():
                state[1] += 1
                tgt = min(len(fills), -(-len(fills) * state[1] // 40))
                while state[0] < tgt:
                    fills[state[0]]()
                    state[0] += 1

            for ci, (i0, ilen) in enumerate(IC):
                if pt == NOTILE - 1:
                    # out-proj fills read ctxT; the pending normalize must
                    # be emitted before they are
                    flush_norm[0]()
                if ci in stage_fills:
                    fills.extend(stage_fills[ci])
                pvbox = [None]

                def emit_pv(item, pvbox=pvbox, ilen=ilen, pt=pt, ci=ci):
                    if pvbox[0] is None:
                        pvbox[0] = psum_pv.tile([P, 1024], F32, tag="pv",
                                                name=f"pv_{pt}_{ci}")
                    pv = pvbox[0]
                    pes, pjt, pjsz = item
                    for hh in range(2):
                        nc.tensor.matmul(
                            pv[:, 512 * hh:512 * hh + ilen],
                            vhead_ap(pjsz, pjt, 2 * pt + hh),
                            pes[:pjsz, 512 * hh:512 * hh + ilen],
                            start=(pjt == 0), stop=(pjt == NSTILE - 1))

                pending = []
                for jt in range(NSTILE):
                    j0, jsz = _stile(jt)
                    sc = psum_sc.tile([P, 1024], F32, tag="sc",
                                      name=f"sc_{pt}_{ci}_{jt}")
                    for hh in range(2):
                        hb = 64 * hh
                        nc.tensor.matmul(
                            sc[:jsz, 512 * hh:512 * hh + ilen],
                            kT[hb:hb + 64, pt, j0:j0 + jsz],
                            qT[hb:hb + 64, pt, i0:i0 + ilen],
                            start=True, stop=True)
                    es = es_pool.tile([P, 1024], BF16, tag="es",
                                      name=f"es_{pt}_{ci}_{jt}")
                    nc.scalar.activation(out=two_run_ap(es, jsz, ilen),
                                         in_=two_run_ap(sc, jsz, ilen),
                                         func=exp_f, scale=scaling)
                    if jt == 1:
                        # lazy normalize of the previous chunk: emitted
                        # after this chunk's first exps so it never
                        # head-of-line blocks the DVE FIFO
                        flush_norm[0]()
                    if pt == NOTILE - 1 and ci == 0 and jt + 3 < NSTILE:
                        vproj_st(pt, jt + 3)
                    else:
                        pace()
                    if len(pending) >= 2:
                        emit_pv(pending.pop(0))
                    pending.append((es, jt, jsz))
                for item in pending:
                    pace()
                    emit_pv(item)

                def norm(pv=pvbox[0], ilen=ilen, i0=i0, pt=pt, ci=ci):
                    # denominator is replicated on psum rows 0-63
                    rec = rec_pool.tile([D, 1024], F32, tag="rec",
                                        name=f"rec_{pt}_{ci}")
                    nc.vector.reciprocal_approx_fast(
                        out=two_run_ap(rec, D, ilen),
                        in_=two_run_ap(pv, D, ilen))
                    for hh in range(2):
                        nc.vector.tensor_mul(
                            ctxT[64 * hh:64 * hh + 64, pt, i0:i0 + ilen],
                            pv[64:128, 512 * hh:512 * hh + ilen],
                            rec[0:64, 512 * hh:512 * hh + ilen])

                def mk_flush(fn):
                    def f():
                        flush_norm[0] = lambda: None
                        fn()
                    return f

                flush_norm[0] = mk_flush(norm)
                pace()
                pace()
            while state[0] < len(fills):
                fills[state[0]]()
                state[0] += 1

        # ---------------- output projection tail ----------------
        flush_norm[0]()
        for it in range(8, NSTILE):
            outproj_it(it)


_NC_CACHE = None


def get_nc():
    global _NC_CACHE
    if _NC_CACHE is None:
        nc = bacc.Bacc(None, target_bir_lowering=False, debug=False)
        _NC_CACHE = build_kernel(nc)
    return _NC_CACHE


def kernel(**inputs):
    from concourse.bass_utils import run_bass_kernel_spmd

    nc = get_nc()
    names = ["hidden_states", "sin", "cos", "Wq", "bq", "Wk", "Wv", "bv", "Wp", "bp"]
    arrs = {k: np.ascontiguousarray(np.asarray(inputs[k], dtype=np.float32))
            for k in names}
    in_maps = []
    for b in range(B):
        m = {k: arrs[k] for k in names if k != "hidden_states"}
        m["hidden_states"] = np.ascontiguousarray(arrs["hidden_states"][b])
        in_maps.append(m)
    res = run_bass_kernel_spmd(nc, in_maps, core_ids=list(range(B)))
    out = np.stack([res.results[b]["out"] for b in range(B)], axis=0)
    return out.astype(np.float32)


if __name__ == "__main__":
    nc = get_nc()
    print("built ok")


# revision 54
# speedup vs baseline: 1.0152x; 1.0152x over previous
"""Dinov3 self-attention Bass kernel for TRN2.

Sharding: data-parallel over batch. B=8 batch elements -> 8 NeuronCores,
one full attention per core, weights replicated. No collectives.

v2 design (HAM-aware): keep the PE stream dense so the HAM clock gate
stays at 8/8 (2.4 GHz).  Per head-pair pt:
  - q/k projections land transposed (qT/kT [d, s]), RoPE'd in place.
  - v lands natural in vsb [j, (12 heads x 64 | 64 ones)]; the 64 ones
    columns make every PV matmul also produce the softmax denominator
    REPLICATED on psum rows 64..127 (M=128 costs the same as M=65).
  - attention per (pt, ic, jt): one [jsz, 1024] psum holds BOTH heads'
    score chunks (h0 cols 0:512, h1 cols 512:1024) -> ONE wide exp on
    the scalar engine -> two PV matmuls accumulate ctx+denom.
  - normalize fully on-chip: DVE reciprocal_approx_fast on the
    replicated denominator rows + DVE multiply -> ctxT bf16.  No DRAM
    bounce, no 4-byte scatter DMAs.
  - projections for pt+1 are interleaved between the QK and PV groups
    of pt's attention so the PE never idles while exp runs; for pt=5
    the first output-projection tiles are the filler.
Engines: PE matmuls; ACT exp (+ tail out-proj evicts); DVE everything
elementwise; GpSimd queue posts the xbar transposes; Sync queue posts
loads/stores.
"""

import contextlib
import sys

import numpy as np

sys.path.insert(0, "/opt/trn_rl_repo")

import concourse.bacc as bacc
import concourse.bass as bass
import concourse.tile as tile
from concourse import mybir

S = 1374
H = 768
NH = 12
D = 64
NROT = 1369
PREFIX = S - NROT  # 5
B = 8

P = 128
NSTILE = (S + P - 1) // P  # 11 s-tiles, last has 94 rows
NOTILE = H // P  # 6 head pairs
SPAD = NSTILE * P  # 1408
IC = ((0, 512), (512, 512), (1024, 350))  # i-chunks, each <= 1 psum bank

F32 = mybir.dt.float32
BF16 = mybir.dt.bfloat16


def _stile(i):
    start = i * P
    return start, min(P, S - start)


def build_kernel(nc):
    x_ext = nc.declare_dram_parameter("hidden_states", [S, H], F32, isOutput=False)
    sin_ext = nc.declare_dram_parameter("sin", [NROT, D], F32, isOutput=False)
    cos_ext = nc.declare_dram_parameter("cos", [NROT, D], F32, isOutput=False)
    wq_ext = nc.declare_dram_parameter("Wq", [H, H], F32, isOutput=False)
    bq_ext = nc.declare_dram_parameter("bq", [H], F32, isOutput=False)
    wk_ext = nc.declare_dram_parameter("Wk", [H, H], F32, isOutput=False)
    wv_ext = nc.declare_dram_parameter("Wv", [H, H], F32, isOutput=False)
    bv_ext = nc.declare_dram_parameter("bv", [H], F32, isOutput=False)
    wp_ext = nc.declare_dram_parameter("Wp", [H, H], F32, isOutput=False)
    bp_ext = nc.declare_dram_parameter("bp", [H], F32, isOutput=False)
    out_ext = nc.declare_dram_parameter("out", [S, H], F32, isOutput=True)

    with tile.TileContext(nc) as tc:
        _body(tc, x_ext, sin_ext, cos_ext, wq_ext, bq_ext, wk_ext,
              wv_ext, bv_ext, wp_ext, bp_ext, out_ext)
    nc.compile()
    return nc


def _body(tc, x_ext, sin_ext, cos_ext, wq_ext, bq_ext, wk_ext, wv_ext,
          bv_ext, wp_ext, bp_ext, out_ext):
    nc = tc.nc

    with contextlib.ExitStack() as ctx:
        persist = ctx.enter_context(tc.tile_pool(name="persist", bufs=1))
        psum_sc = ctx.enter_context(tc.tile_pool(name="psum_sc", bufs=2, space="PSUM"))
        psum_pv = ctx.enter_context(tc.tile_pool(name="psum_pv", bufs=1, space="PSUM"))
        psum_pj = ctx.enter_context(tc.tile_pool(name="psum_pj", bufs=2, space="PSUM"))
        es_pool = ctx.enter_context(tc.tile_pool(name="es_pool", bufs=4))
        rec_pool = ctx.enter_context(tc.tile_pool(name="rec_pool", bufs=2))
        ropet = ctx.enter_context(tc.tile_pool(name="ropet", bufs=2))
        stage = ctx.enter_context(tc.tile_pool(name="stage", bufs=2))
        cstage = ctx.enter_context(tc.tile_pool(name="cstage", bufs=1))
        outst = ctx.enter_context(tc.tile_pool(name="outst", bufs=2))

        xT = persist.tile([P, NOTILE, SPAD], BF16)     # xT[p, t, s] = x[s, 128t+p]
        qT = persist.tile([P, NOTILE, SPAD], BF16)     # roped q, [(hh,d), pt, s]
        kT = persist.tile([P, NOTILE, SPAD], BF16)
        ctxT = persist.tile([P, NOTILE, SPAD], BF16)   # normalized ctx^T
        # per head: 64 ones columns + 64 v columns, so every PV matmul
        # (lhsT = one head slot, M=128) also produces the denominator
        # replicated on psum rows 0-63 (ctx on rows 64-127).  The ones
        # block comes FIRST so the reciprocal reads psum at partition
        # offset 0 — HW lowering drops nonzero partition offsets on the
        # custom-DVE reciprocal input.
        vsb = persist.tile([P, NSTILE, NH, 2 * D], BF16)
        wqT = persist.tile([P, NOTILE, H], BF16)
        wkT = persist.tile([P, NOTILE, H], BF16)
        wvT = persist.tile([P, NOTILE, H], BF16)
        wpT = persist.tile([P, NOTILE, H], BF16)
        cc2 = persist.tile([P, SPAD], BF16)            # cos^T stacked twice
        ss2 = persist.tile([P, SPAD], BF16)            # sin^T stacked, sign-baked
        bq_sb = persist.tile([P, NOTILE], F32)
        bv_row = persist.tile([1, H], BF16)
        bp_row = persist.tile([1, H], BF16)
        ones_row = persist.tile([1, P], BF16)
        ident = persist.tile([P, P], BF16)

        nc.vector.memset(ones_row, 1.0)
        from concourse.masks import make_identity
        make_identity(nc, ident)

        # ---------------- load helpers ----------------
        def load_biases():
            nc.sync.dma_start(out=bq_sb, in_=bq_ext.rearrange("(t p) -> p t", p=P))
            # 1-lane casts are ~6x faster on ACT than DVE, and the DVE
            # queue feeds the startup-critical x casts
            bstage = stage.tile([1, H], F32, tag="bias_stage", bufs=1)
            nc.sync.dma_start(out=bstage, in_=bv_ext.rearrange("(a h) -> a h", a=1))
            nc.scalar.copy(out=bv_row, in_=bstage)
            bstage2 = stage.tile([1, H], F32, tag="bias_stage2", bufs=1)
            nc.sync.dma_start(out=bstage2, in_=bp_ext.rearrange("(a h) -> a h", a=1))
            nc.scalar.copy(out=bp_row, in_=bstage2)

        def load_sincos():
            # [NROT, 64] f32 -> bf16 [64, s] via one batched xbar transpose,
            # then copy into both halves of [128, s]; bake rotate_half signs.
            n_rtile = (NROT + P - 1) // P
            for src_ext, dstT in ((cos_ext, cc2), (sin_ext, ss2)):
                cst_all = cstage.tile([P, SPAD], F32, tag="cs_stage")
                csb_all = cstage.tile([P, SPAD], BF16, tag="cs_stage_bf")
                csT3 = cstage.tile([P, n_rtile, P], BF16, tag="cs_T3")
                nc.gpsimd.memset(csb_all, 0.0)
                for i in range(n_rtile):
                    r0 = i * P
                    rsz = min(P, NROT - r0)
                    nc.sync.dma_start(out=cst_all[:rsz, i * P:i * P + D],
                                      in_=src_ext[r0:r0 + rsz, :])
                    nc.vector.tensor_copy(
                        out=csb_all[:rsz, i * P:i * P + D],
                        in_=cst_all[:rsz, i * P:i * P + D])
                nc.scalar.dma_start_transpose(out=csT3, in_=csb_all)
                # scatter copies on ACT: the scalar queue is empty during
                # the prologue while DVE feeds the startup-critical x casts
                for i in range(n_rtile):
                    r0 = i * P
                    rsz = min(P, NROT - r0)
                    for half in range(2):
                        nc.scalar.copy(
                            out=dstT[64 * half:64 * half + 64, r0:r0 + rsz],
                            in_=csT3[0:D, i, :rsz])
            for base in (0, 64):
                sl = slice(base, base + 32)
                nc.vector.tensor_scalar_mul(ss2[sl, :NROT], ss2[sl, :NROT], -1.0)

        _xevict = []

        def load_x_tile(st):
            # transpose on the PE (idle at startup) — the xbar path costs
            # a 1.25us scalar-queue post per tile plus sem-chained stage
            # slots, which serialized the whole startup at ~6us/tile.
            # The psum evict is deferred one tile so it never head-of-line
            # blocks the next tile's cast on the DVE FIFO.
            s0, ssz = _stile(st)
            xs = stage.tile([P, H], F32, tag="x_stage", name=f"xs_{st}")
            xb = stage.tile([P, H], BF16, tag="x_stage_bf", name=f"xb_{st}")
            if ssz < P:
                nc.vector.memset(xb, 0.0)
            nc.sync.dma_start(out=xs[:ssz], in_=x_ext[s0:s0 + ssz, :])
            nc.vector.tensor_copy(out=xb[:ssz], in_=xs[:ssz])
            xtp = psum_sc.tile([P, H], BF16, tag="sc", name=f"xtp_{st}")
            for kt in range(NOTILE):
                nc.tensor.transpose(xtp[:, kt * P:(kt + 1) * P],
                                    xb[:, kt * P:(kt + 1) * P], ident)
            _xevict.append((xtp, s0))
            if len(_xevict) > 1:
                flush_xevict(1)

        def flush_xevict(keep=0):
            while len(_xevict) > keep:
                xtp, s0 = _xevict.pop(0)
                nc.vector.tensor_copy(
                    out=xT[:, :, s0:s0 + P],
                    in_=xtp.rearrange("p (k q) -> p k q", q=P))

        def load_w_row(w_ext, wT, r, tq=None):
            # tq: engine queue for the xbar-transpose post.  The scalar
            # queue feeds the exps, so only startup-critical transposes
            # go there; the rest post from the (idle) sync queue.
            ws = stage.tile([P, H], F32, tag="w_stage", name=f"ws_{wT.name}_{r}")
            wb = stage.tile([P, H], BF16, tag="w_stage_bf", name=f"wb_{wT.name}_{r}")
            nc.sync.dma_start(out=ws, in_=w_ext[r * P:(r + 1) * P, :])
            nc.vector.tensor_copy(out=wb, in_=ws)
            (tq or nc.scalar).dma_start_transpose(
                out=wT[:, :, r * P:(r + 1) * P], in_=wb)

        # ---------------- projection emit-units ----------------
        _pj_live = {}

        def qkproj_half(wT, dst, ot, ci, bias, half, act_evict=False):
            # half 0: kts 0-2 (allocates psum); half 1: kts 3-5 + evict.
            # Split so paced filling can interleave at ~0.6us granularity.
            # act_evict: in the prologue ACT is idle and the DVE FIFO feeds
            # the startup-critical x casts — evict there instead.
            i0, ilen = IC[ci]
            key = (wT.name, ot, ci)
            if half == 0:
                _pj_live[key] = psum_pj.tile(
                    [P, 512], F32, tag="pj",
                    name=f"qk_{dst.name}_{ot}_{ci}")[:, :ilen]
            pj = _pj_live[key]
            for kt in range(3 * half, 3 * half + 3):
                nc.tensor.matmul(
                    pj, wT[:, kt, ot * P:(ot + 1) * P],
                    xT[:, kt, i0:i0 + ilen],
                    start=(kt == 0), stop=(kt == NOTILE - 1))
            if half == 1:
                del _pj_live[key]
                if bias:
                    if act_evict:
                        nc.scalar.add(dst[:, ot, i0:i0 + ilen], pj,
                                      bq_sb[:, ot:ot + 1])
                    else:
                        nc.vector.tensor_scalar_add(dst[:, ot, i0:i0 + ilen],
                                                    pj, bq_sb[:, ot:ot + 1])
                elif act_evict:
                    nc.scalar.copy(out=dst[:, ot, i0:i0 + ilen], in_=pj)
                else:
                    nc.vector.tensor_copy(out=dst[:, ot, i0:i0 + ilen], in_=pj)

        _rope_live = {}

        def rope_dma(dst, ot):
            rot = ropet.tile([P, NROT], BF16, tag="rot", name=f"rot_{dst.name}_{ot}")
            _rope_live[(dst.name, ot)] = rot
            sl = slice(PREFIX, PREFIX + NROT)
            for (dst0, src0) in ((0, 32), (32, 0), (64, 96), (96, 64)):
                nc.sync.dma_start(
                    out=rot[dst0:dst0 + 32, :],
                    in_=dst[src0:src0 + 32, ot, sl])

        def rope_mul(dst, ot):
            # separate unit: the in-place mul WAR-waits on the rotate DMAs;
            # emitting it later keeps that wait off the DVE FIFO head
            rot = _rope_live.pop((dst.name, ot))
            sl = slice(PREFIX, PREFIX + NROT)
            nc.vector.tensor_mul(dst[:, ot, sl], dst[:, ot, sl], cc2[:, :NROT])
            nc.vector.tensor_mul(rot, rot, ss2[:, :NROT])
            nc.vector.tensor_add(dst[:, ot, sl], dst[:, ot, sl], rot)

        def rope(dst, ot):
            rope_dma(dst, ot)
            rope_mul(dst, ot)

        def vproj_st(pt, st):
            s0, ssz = _stile(st)
            pj = psum_pj.tile([P, 512], F32, tag="pj",
                              name=f"v_{pt}_{st}")[:, :P]
            for kt in range(NOTILE):
                nc.tensor.matmul(
                    pj[:ssz, :], xT[:, kt, s0:s0 + ssz],
                    wvT[:, kt, pt * P:(pt + 1) * P],
                    start=(kt == 0), stop=False)
            nc.tensor.matmul(
                pj[:ssz, :], ones_row[:, :ssz],
                bv_row[:, pt * P:(pt + 1) * P],
                start=False, stop=True)
            nc.vector.tensor_copy(
                out=vsb[:ssz, st, 2 * pt:2 * pt + 2, D:2 * D],
                in_=pj[:ssz, :].rearrange("p (h d) -> p h d", d=D))

        def outproj_it(it):
            s0, ssz = _stile(it)
            ot_t = outst.tile([P, H], F32, tag="ostage", name=f"ost_{it}")
            for ci, (o0, on) in enumerate(((0, 512), (512, 256))):
                pj = psum_pj.tile([P, 512], F32, tag="pj",
                                  name=f"o_{it}_{ci}")[:, :on]
                for kt in range(NOTILE):
                    nc.tensor.matmul(
                        pj[:ssz, :], ctxT[:, kt, s0:s0 + ssz],
                        wpT[:, kt, o0:o0 + on],
                        start=(kt == 0), stop=False)
                nc.tensor.matmul(
                    pj[:ssz, :], ones_row[:, :ssz], bp_row[:, o0:o0 + on],
                    start=False, stop=True)
                nc.scalar.copy(out=ot_t[:ssz, o0:o0 + on], in_=pj[:ssz, :])
            nc.sync.dma_start(out=out_ext[s0:s0 + ssz, :], in_=ot_t[:ssz])

        # ---------------- emission order ----------------
        # wave 1: exactly what the first projection matmuls need (x tiles
        # 0-3 + row 0 of Wq/Wk/Wv), projection chunks interleaved with the
        # remaining x tiles so the PE queue pipelines through the startup.
        for st in range(4):
            load_x_tile(st)
        load_w_row(wq_ext, wqT, 0)
        load_w_row(wk_ext, wkT, 0)
        load_w_row(wv_ext, wvT, 0)
        load_biases()
        flush_xevict()
        qkproj_half(wqT, qT, 0, 0, True, 0)
        qkproj_half(wqT, qT, 0, 0, True, 1)
        for st in range(4, 8):
            load_x_tile(st)
        flush_xevict()
        qkproj_half(wqT, qT, 0, 1, True, 0)
        qkproj_half(wqT, qT, 0, 1, True, 1)
        for st in range(8, NSTILE):
            load_x_tile(st)
        flush_xevict()
        qkproj_half(wqT, qT, 0, 2, True, 0)
        qkproj_half(wqT, qT, 0, 2, True, 1)
        # sincos after all x tiles: its 22 staging DMAs would otherwise
        # block the x loads on the sync queue (cos->sin stage-tile WAR)
        load_sincos()
        nc.gpsimd.memset(vsb[:, :, :, 0:D], 1.0)
        for ci in range(3):
            qkproj_half(wkT, kT, 0, ci, False, 0)
            qkproj_half(wkT, kT, 0, ci, False, 1)
        rope(qT, 0)
        rope(kT, 0)
        for st in range(NSTILE):
            vproj_st(0, st)

        # row 1 of each weight feeds proj(1), the attention(0) filler
        load_w_row(wq_ext, wqT, 1, tq=nc.sync)
        load_w_row(wk_ext, wkT, 1, tq=nc.sync)
        load_w_row(wv_ext, wvT, 1, tq=nc.sync)

        def vhead_ap(jsz, jt, h):
            return vsb[:jsz, jt, h, :]

        def two_run_ap(t, rows, ilen):
            """[rows, 2, ilen] AP over a [P, 1024] tile: cols {0:ilen} and
            {512:512+ilen} — skips the unwritten hole when ilen < 512.
            For full-width chunks a flat 2D AP is equivalent and cheaper."""
            s = t[:rows, :]
            if ilen == 512:
                return s
            dims = [list(d) for d in s.ap]
            st = dims[-1][0]
            return bass.AP(tensor=s.tensor, offset=s.offset,
                           ap=[dims[0], [512 * st, 2], [st, ilen]])

        exp_f = mybir.ActivationFunctionType.Exp
        scaling = float(D) ** -0.5
        flush_norm = [lambda: None]

        for pt in range(NOTILE):
            # filler units: projections for pt+1 (for pt=4: only the first
            # 3 v-proj tiles — the rest fill attention(5, ic0) itself),
            # Wp loads during attention(0), out-proj row-tiles for pt=5.
            # just-in-time weight streaming: row pt+2 of Wq/Wk/Wv (feeds
            # proj(pt+2)) and one Wp row per pt — spread so no queue ever
            # sees a burst of weight traffic.
            fills = []
            if pt + 2 < NOTILE:
                for w_ext, wT in ((wq_ext, wqT), (wk_ext, wkT), (wv_ext, wvT)):
                    fills.append(lambda w_ext=w_ext, wT=wT:
                                 load_w_row(w_ext, wT, pt + 2, tq=nc.sync))
            if pt < NOTILE - 1:
                fills.append(lambda pt=pt: load_w_row(wp_ext, wpT, pt, tq=nc.sync))
                if pt == NOTILE - 2:
                    fills.append(lambda: load_w_row(wp_ext, wpT, NOTILE - 1,
                                                    tq=nc.sync))
            if pt + 1 < NOTILE:
                np1 = pt + 1
                for ci3 in range(3):
                    for half in range(2):
                        fills.append(lambda ci3=ci3, half=half, np1=np1:
                                     qkproj_half(wqT, qT, np1, ci3, True, half))
                fills.append(lambda np1=np1: rope_dma(qT, np1))
                for ci3 in range(3):
                    for half in range(2):
                        fills.append(lambda ci3=ci3, half=half, np1=np1:
                                     qkproj_half(wkT, kT, np1, ci3, False, half))
                fills.append(lambda np1=np1: rope_dma(kT, np1))
                fills.append(lambda np1=np1: rope_mul(qT, np1))
                vmax = NSTILE if np1 < NOTILE - 1 else 3
                for st in range(vmax):
                    if st == 1:
                        fills.append(lambda np1=np1: rope_mul(kT, np1))
                    fills.append(lambda st=st, np1=np1: vproj_st(np1, st))
            # (for pt=5 the rest of v-proj(5) is emitted inline in the ic0
            # jt loop below — emission order must stay ahead of the PV
            # reads, since Tile tracks dependencies in trace order.)
            stage_fills = {}
            if pt == NOTILE - 1:
                # it 0-3 need ctxT i cols 0:512 (ready after ic0's
                # normalize); it 4-7 need cols up to 1024 (after ic1).
                stage_fills[1] = [lambda it=it: outproj_it(it) for it in range(4)]
                stage_fills[2] = [lambda it=it: outproj_it(it) for it in range(4, 8)]

            state = [0, 0]  # units emitted, paces done (of 39)

            def pace():
                state[1] += 1
                tgt = min(len(fills), -(-len(fills) * state[1] // 45))
                while state[0] < tgt:
                    fills[state[0]]()
                    state[0] += 1

            for ci, (i0, ilen) in enumerate(IC):
                if pt == NOTILE - 1:
                    # out-proj fills read ctxT; the pending normalize must
                    # be emitted before they are
                    flush_norm[0]()
                if ci in stage_fills:
                    fills.extend(stage_fills[ci])
                pvbox = [None]

                def emit_pv(item, pvbox=pvbox, ilen=ilen, pt=pt, ci=ci):
                    if pvbox[0] is None:
                        pvbox[0] = psum_pv.tile([P, 1024], F32, tag="pv",
                                                name=f"pv_{pt}_{ci}")
                    pv = pvbox[0]
                    pes, pjt, pjsz = item
                    for hh in range(2):
                        nc.tensor.matmul(
                            pv[:, 512 * hh:512 * hh + ilen],
                            vhead_ap(pjsz, pjt, 2 * pt + hh),
                            pes[:pjsz, 512 * hh:512 * hh + ilen],
                            start=(pjt == 0), stop=(pjt == NSTILE - 1))

                pending = []
                for jt in range(NSTILE):
                    j0, jsz = _stile(jt)
                    sc = psum_sc.tile([P, 1024], F32, tag="sc",
                                      name=f"sc_{pt}_{ci}_{jt}")
                    for hh in range(2):
                        hb = 64 * hh
                        nc.tensor.matmul(
                            sc[:jsz, 512 * hh:512 * hh + ilen],
                            kT[hb:hb + 64, pt, j0:j0 + jsz],
                            qT[hb:hb + 64, pt, i0:i0 + ilen],
                            start=True, stop=True)
                    es = es_pool.tile([P, 1024], BF16, tag="es",
                                      name=f"es_{pt}_{ci}_{jt}")
                    nc.scalar.activation(out=two_run_ap(es, jsz, ilen),
                                         in_=two_run_ap(sc, jsz, ilen),
                                         func=exp_f, scale=scaling)
                    if jt == 1:
                        # lazy normalize of the previous chunk: emitted
                        # after this chunk's first exps so it never
                        # head-of-line blocks the DVE FIFO
                        flush_norm[0]()
                    if pt == NOTILE - 1 and ci == 0 and jt + 3 < NSTILE:
                        vproj_st(pt, jt + 3)
                    else:
                        pace()
                    if len(pending) >= 2:
                        emit_pv(pending.pop(0))
                    pending.append((es, jt, jsz))
                for item in pending:
                    pace()
                    emit_pv(item)

                def norm(pv=pvbox[0], ilen=ilen, i0=i0, pt=pt, ci=ci):
                    # denominator is replicated on psum rows 0-63
                    rec = rec_pool.tile([D, 1024], F32, tag="rec",
                                        name=f"rec_{pt}_{ci}")
                    nc.vector.reciprocal_approx_fast(
                        out=two_run_ap(rec, D, ilen),
                        in_=two_run_ap(pv, D, ilen))
                    for hh in range(2):
                        nc.vector.tensor_mul(
                            ctxT[64 * hh:64 * hh + 64, pt, i0:i0 + ilen],
                            pv[64:128, 512 * hh:512 * hh + ilen],
                            rec[0:64, 512 * hh:512 * hh + ilen])

                def mk_flush(fn):
                    def f():
                        flush_norm[0] = lambda: None
                        fn()
                    return f

                flush_norm[0] = mk_flush(norm)
                pace()
                pace()
            while state[0] < len(fills):
                fills[state[0]]()
                state[0] += 1

        # ---------------- output projection tail ----------------
        flush_norm[0]()
        for it in range(8, NSTILE):
            outproj_it(it)


_NC_CACHE = None


def get_nc():
    global _NC_CACHE
    if _NC_CACHE is None:
        nc = bacc.Bacc(None, target_bir_lowering=False, debug=False)
        _NC_CACHE = build_kernel(nc)
    return _NC_CACHE


def kernel(**inputs):
    from concourse.bass_utils import run_bass_kernel_spmd

    nc = get_nc()
    names = ["hidden_states", "sin", "cos", "Wq", "bq", "Wk", "Wv", "bv", "Wp", "bp"]
    arrs = {k: np.ascontiguousarray(np.asarray(inputs[k], dtype=np.float32))
            for k in names}
    in_maps = []
    for b in range(B):
        m = {k: arrs[k] for k in names if k != "hidden_states"}
        m["hidden_states"] = np.ascontiguousarray(arrs["hidden_states"][b])
        in_maps.append(m)
    res = run_bass_kernel_spmd(nc, in_maps, core_ids=list(range(B)))
    out = np.stack([res.results[b]["out"] for b in range(B)], axis=0)
    return out.astype(np.float32)


if __name__ == "__main__":
    nc = get_nc()
    print("built ok")


# revision 55
# speedup vs baseline: 1.0444x; 1.0287x over previous
"""Dinov3 self-attention Bass kernel for TRN2.

Sharding: data-parallel over batch. B=8 batch elements -> 8 NeuronCores,
one full attention per core, weights replicated. No collectives.

v2 design (HAM-aware): keep the PE stream dense so the HAM clock gate
stays at 8/8 (2.4 GHz).  Per head-pair pt:
  - q/k projections land transposed (qT/kT [d, s]), RoPE'd in place.
  - v lands natural in vsb [j, (12 heads x 64 | 64 ones)]; the 64 ones
    columns make every PV matmul also produce the softmax denominator
    REPLICATED on psum rows 64..127 (M=128 costs the same as M=65).
  - attention per (pt, ic, jt): one [jsz, 1024] psum holds BOTH heads'
    score chunks (h0 cols 0:512, h1 cols 512:1024) -> ONE wide exp on
    the scalar engine -> two PV matmuls accumulate ctx+denom.
  - normalize fully on-chip: DVE reciprocal_approx_fast on the
    replicated denominator rows + DVE multiply -> ctxT bf16.  No DRAM
    bounce, no 4-byte scatter DMAs.
  - projections for pt+1 are interleaved between the QK and PV groups
    of pt's attention so the PE never idles while exp runs; for pt=5
    the first output-projection tiles are the filler.
Engines: PE matmuls; ACT exp (+ tail out-proj evicts); DVE everything
elementwise; GpSimd queue posts the xbar transposes; Sync queue posts
loads/stores.
"""

import contextlib
import sys

import numpy as np

sys.path.insert(0, "/opt/trn_rl_repo")

import concourse.bacc as bacc
import concourse.bass as bass
import concourse.tile as tile
from concourse import mybir

S = 1374
H = 768
NH = 12
D = 64
NROT = 1369
PREFIX = S - NROT  # 5
B = 8

P = 128
NSTILE = (S + P - 1) // P  # 11 s-tiles, last has 94 rows
NOTILE = H // P  # 6 head pairs
SPAD = NSTILE * P  # 1408
IC = ((0, 512), (512, 512), (1024, 350))  # i-chunks, each <= 1 psum bank

F32 = mybir.dt.float32
BF16 = mybir.dt.bfloat16


def _stile(i):
    start = i * P
    return start, min(P, S - start)


def build_kernel(nc):
    x_ext = nc.declare_dram_parameter("hidden_states", [S, H], F32, isOutput=False)
    sin_ext = nc.declare_dram_parameter("sin", [NROT, D], F32, isOutput=False)
    cos_ext = nc.declare_dram_parameter("cos", [NROT, D], F32, isOutput=False)
    wq_ext = nc.declare_dram_parameter("Wq", [H, H], F32, isOutput=False)
    bq_ext = nc.declare_dram_parameter("bq", [H], F32, isOutput=False)
    wk_ext = nc.declare_dram_parameter("Wk", [H, H], F32, isOutput=False)
    wv_ext = nc.declare_dram_parameter("Wv", [H, H], F32, isOutput=False)
    bv_ext = nc.declare_dram_parameter("bv", [H], F32, isOutput=False)
    wp_ext = nc.declare_dram_parameter("Wp", [H, H], F32, isOutput=False)
    bp_ext = nc.declare_dram_parameter("bp", [H], F32, isOutput=False)
    out_ext = nc.declare_dram_parameter("out", [S, H], F32, isOutput=True)

    with tile.TileContext(nc) as tc:
        _body(tc, x_ext, sin_ext, cos_ext, wq_ext, bq_ext, wk_ext,
              wv_ext, bv_ext, wp_ext, bp_ext, out_ext)
    nc.compile()
    return nc


def _body(tc, x_ext, sin_ext, cos_ext, wq_ext, bq_ext, wk_ext, wv_ext,
          bv_ext, wp_ext, bp_ext, out_ext):
    nc = tc.nc

    with contextlib.ExitStack() as ctx:
        persist = ctx.enter_context(tc.tile_pool(name="persist", bufs=1))
        psum_sc = ctx.enter_context(tc.tile_pool(name="psum_sc", bufs=2, space="PSUM"))
        psum_pv = ctx.enter_context(tc.tile_pool(name="psum_pv", bufs=1, space="PSUM"))
        psum_pj = ctx.enter_context(tc.tile_pool(name="psum_pj", bufs=2, space="PSUM"))
        es_pool = ctx.enter_context(tc.tile_pool(name="es_pool", bufs=4))
        rec_pool = ctx.enter_context(tc.tile_pool(name="rec_pool", bufs=2))
        ropet = ctx.enter_context(tc.tile_pool(name="ropet", bufs=2))
        stage = ctx.enter_context(tc.tile_pool(name="stage", bufs=2))
        cstage = ctx.enter_context(tc.tile_pool(name="cstage", bufs=1))
        outst = ctx.enter_context(tc.tile_pool(name="outst", bufs=2))

        xT = persist.tile([P, NOTILE, SPAD], BF16)     # xT[p, t, s] = x[s, 128t+p]
        qT = persist.tile([P, NOTILE, SPAD], BF16)     # roped q, [(hh,d), pt, s]
        kT = persist.tile([P, NOTILE, SPAD], BF16)
        ctxT = persist.tile([P, NOTILE, SPAD], BF16)   # normalized ctx^T
        # per head: 64 ones columns + 64 v columns, so every PV matmul
        # (lhsT = one head slot, M=128) also produces the denominator
        # replicated on psum rows 0-63 (ctx on rows 64-127).  The ones
        # block comes FIRST so the reciprocal reads psum at partition
        # offset 0 — HW lowering drops nonzero partition offsets on the
        # custom-DVE reciprocal input.
        vsb = persist.tile([P, NSTILE, NH, 2 * D], BF16)
        wqT = persist.tile([P, NOTILE, H], BF16)
        wkT = persist.tile([P, NOTILE, H], BF16)
        wvT = persist.tile([P, NOTILE, H], BF16)
        wpT = persist.tile([P, NOTILE, H], BF16)
        cc2 = persist.tile([P, SPAD], BF16)            # cos^T stacked twice
        ss2 = persist.tile([P, SPAD], BF16)            # sin^T stacked, sign-baked
        bq_sb = persist.tile([P, NOTILE], F32)
        bv_row = persist.tile([1, H], BF16)
        bp_row = persist.tile([1, H], BF16)
        ones_row = persist.tile([1, P], BF16)
        ident = persist.tile([P, P], BF16)

        nc.vector.memset(ones_row, 1.0)
        from concourse.masks import make_identity
        make_identity(nc, ident)

        # ---------------- load helpers ----------------
        def load_biases():
            nc.sync.dma_start(out=bq_sb, in_=bq_ext.rearrange("(t p) -> p t", p=P))
            # 1-lane casts are ~6x faster on ACT than DVE, and the DVE
            # queue feeds the startup-critical x casts
            bstage = stage.tile([1, H], F32, tag="bias_stage", bufs=1)
            nc.sync.dma_start(out=bstage, in_=bv_ext.rearrange("(a h) -> a h", a=1))
            nc.scalar.copy(out=bv_row, in_=bstage)
            bstage2 = stage.tile([1, H], F32, tag="bias_stage2", bufs=1)
            nc.sync.dma_start(out=bstage2, in_=bp_ext.rearrange("(a h) -> a h", a=1))
            nc.scalar.copy(out=bp_row, in_=bstage2)

        def load_sincos():
            # [NROT, 64] f32 -> bf16 [64, s] via one batched xbar transpose,
            # then copy into both halves of [128, s]; bake rotate_half signs.
            n_rtile = (NROT + P - 1) // P
            for src_ext, dstT in ((cos_ext, cc2), (sin_ext, ss2)):
                cst_all = cstage.tile([P, SPAD], F32, tag="cs_stage")
                csb_all = cstage.tile([P, SPAD], BF16, tag="cs_stage_bf")
                csT3 = cstage.tile([P, n_rtile, P], BF16, tag="cs_T3")
                nc.gpsimd.memset(csb_all, 0.0)
                for i in range(n_rtile):
                    r0 = i * P
                    rsz = min(P, NROT - r0)
                    nc.sync.dma_start(out=cst_all[:rsz, i * P:i * P + D],
                                      in_=src_ext[r0:r0 + rsz, :])
                    nc.vector.tensor_copy(
                        out=csb_all[:rsz, i * P:i * P + D],
                        in_=cst_all[:rsz, i * P:i * P + D])
                nc.scalar.dma_start_transpose(out=csT3, in_=csb_all)
                # scatter copies on ACT: the scalar queue is empty during
                # the prologue while DVE feeds the startup-critical x casts
                for i in range(n_rtile):
                    r0 = i * P
                    rsz = min(P, NROT - r0)
                    for half in range(2):
                        nc.scalar.copy(
                            out=dstT[64 * half:64 * half + 64, r0:r0 + rsz],
                            in_=csT3[0:D, i, :rsz])
            for base in (0, 64):
                sl = slice(base, base + 32)
                nc.vector.tensor_scalar_mul(ss2[sl, :NROT], ss2[sl, :NROT], -1.0)

        _xevict = []

        def load_x_tile(st):
            # transpose on the PE (idle at startup) — the xbar path costs
            # a 1.25us scalar-queue post per tile plus sem-chained stage
            # slots, which serialized the whole startup at ~6us/tile.
            # The psum evict is deferred one tile so it never head-of-line
            # blocks the next tile's cast on the DVE FIFO.
            s0, ssz = _stile(st)
            xs = stage.tile([P, H], F32, tag="x_stage", name=f"xs_{st}")
            xb = stage.tile([P, H], BF16, tag="x_stage_bf", name=f"xb_{st}")
            if ssz < P:
                nc.vector.memset(xb, 0.0)
            nc.sync.dma_start(out=xs[:ssz], in_=x_ext[s0:s0 + ssz, :])
            nc.vector.tensor_copy(out=xb[:ssz], in_=xs[:ssz])
            xtp = psum_sc.tile([P, H], BF16, tag="sc", name=f"xtp_{st}")
            for kt in range(NOTILE):
                nc.tensor.transpose(xtp[:, kt * P:(kt + 1) * P],
                                    xb[:, kt * P:(kt + 1) * P], ident)
            _xevict.append((xtp, s0))
            if len(_xevict) > 1:
                flush_xevict(1)

        def flush_xevict(keep=0):
            while len(_xevict) > keep:
                xtp, s0 = _xevict.pop(0)
                nc.vector.tensor_copy(
                    out=xT[:, :, s0:s0 + P],
                    in_=xtp.rearrange("p (k q) -> p k q", q=P))

        def load_w_row(w_ext, wT, r, tq=None):
            # tq: engine queue for the xbar-transpose post.  The scalar
            # queue feeds the exps, so only startup-critical transposes
            # go there; the rest post from the (idle) sync queue.
            ws = stage.tile([P, H], F32, tag="w_stage", name=f"ws_{wT.name}_{r}")
            wb = stage.tile([P, H], BF16, tag="w_stage_bf", name=f"wb_{wT.name}_{r}")
            nc.sync.dma_start(out=ws, in_=w_ext[r * P:(r + 1) * P, :])
            nc.vector.tensor_copy(out=wb, in_=ws)
            (tq or nc.scalar).dma_start_transpose(
                out=wT[:, :, r * P:(r + 1) * P], in_=wb)

        # ---------------- projection emit-units ----------------
        _pj_live = {}

        def qkproj_half(wT, dst, ot, ci, bias, half, act_evict=False):
            # half 0: kts 0-2 (allocates psum); half 1: kts 3-5 + evict.
            # Split so paced filling can interleave at ~0.6us granularity.
            # act_evict: in the prologue ACT is idle and the DVE FIFO feeds
            # the startup-critical x casts — evict there instead.
            i0, ilen = IC[ci]
            key = (wT.name, ot, ci)
            if half == 0:
                _pj_live[key] = psum_pj.tile(
                    [P, 512], F32, tag="pj",
                    name=f"qk_{dst.name}_{ot}_{ci}")[:, :ilen]
            pj = _pj_live[key]
            for kt in range(3 * half, 3 * half + 3):
                nc.tensor.matmul(
                    pj, wT[:, kt, ot * P:(ot + 1) * P],
                    xT[:, kt, i0:i0 + ilen],
                    start=(kt == 0), stop=(kt == NOTILE - 1))
            if half == 1:
                del _pj_live[key]
                if bias:
                    if act_evict:
                        nc.scalar.add(dst[:, ot, i0:i0 + ilen], pj,
                                      bq_sb[:, ot:ot + 1])
                    else:
                        nc.vector.tensor_scalar_add(dst[:, ot, i0:i0 + ilen],
                                                    pj, bq_sb[:, ot:ot + 1])
                elif act_evict:
                    nc.scalar.copy(out=dst[:, ot, i0:i0 + ilen], in_=pj)
                else:
                    nc.vector.tensor_copy(out=dst[:, ot, i0:i0 + ilen], in_=pj)

        _rope_live = {}

        def rope_dma(dst, ot):
            rot = ropet.tile([P, NROT], BF16, tag="rot", name=f"rot_{dst.name}_{ot}")
            _rope_live[(dst.name, ot)] = rot
            sl = slice(PREFIX, PREFIX + NROT)
            for (dst0, src0) in ((0, 32), (32, 0), (64, 96), (96, 64)):
                nc.sync.dma_start(
                    out=rot[dst0:dst0 + 32, :],
                    in_=dst[src0:src0 + 32, ot, sl])

        def rope_mul(dst, ot):
            # separate unit: the in-place mul WAR-waits on the rotate DMAs;
            # emitting it later keeps that wait off the DVE FIFO head
            rot = _rope_live.pop((dst.name, ot))
            sl = slice(PREFIX, PREFIX + NROT)
            nc.vector.tensor_mul(dst[:, ot, sl], dst[:, ot, sl], cc2[:, :NROT])
            nc.vector.tensor_mul(rot, rot, ss2[:, :NROT])
            nc.vector.tensor_add(dst[:, ot, sl], dst[:, ot, sl], rot)

        def rope(dst, ot):
            rope_dma(dst, ot)
            rope_mul(dst, ot)

        def vproj_st(pt, st):
            s0, ssz = _stile(st)
            pj = psum_pj.tile([P, 512], F32, tag="pj",
                              name=f"v_{pt}_{st}")[:, :P]
            for kt in range(NOTILE):
                nc.tensor.matmul(
                    pj[:ssz, :], xT[:, kt, s0:s0 + ssz],
                    wvT[:, kt, pt * P:(pt + 1) * P],
                    start=(kt == 0), stop=False)
            nc.tensor.matmul(
                pj[:ssz, :], ones_row[:, :ssz],
                bv_row[:, pt * P:(pt + 1) * P],
                start=False, stop=True)
            nc.vector.tensor_copy(
                out=vsb[:ssz, st, 2 * pt:2 * pt + 2, D:2 * D],
                in_=pj[:ssz, :].rearrange("p (h d) -> p h d", d=D))

        def outproj_it(it):
            s0, ssz = _stile(it)
            ot_t = outst.tile([P, H], F32, tag="ostage", name=f"ost_{it}")
            for ci, (o0, on) in enumerate(((0, 512), (512, 256))):
                pj = psum_pj.tile([P, 512], F32, tag="pj",
                                  name=f"o_{it}_{ci}")[:, :on]
                for kt in range(NOTILE):
                    nc.tensor.matmul(
                        pj[:ssz, :], ctxT[:, kt, s0:s0 + ssz],
                        wpT[:, kt, o0:o0 + on],
                        start=(kt == 0), stop=False)
                nc.tensor.matmul(
                    pj[:ssz, :], ones_row[:, :ssz], bp_row[:, o0:o0 + on],
                    start=False, stop=True)
                nc.scalar.copy(out=ot_t[:ssz, o0:o0 + on], in_=pj[:ssz, :])
            nc.sync.dma_start(out=out_ext[s0:s0 + ssz, :], in_=ot_t[:ssz])

        # ---------------- emission order ----------------
        # wave 1: exactly what the first projection matmuls need (x tiles
        # 0-3 + row 0 of Wq/Wk/Wv), projection chunks interleaved with the
        # remaining x tiles so the PE queue pipelines through the startup.
        for st in range(4):
            load_x_tile(st)
        load_w_row(wq_ext, wqT, 0)
        load_w_row(wk_ext, wkT, 0)
        load_w_row(wv_ext, wvT, 0)
        load_biases()
        # all x tiles before any projection chunk: the projection evicts
        # would otherwise head-of-line block the x casts on the DVE FIFO
        for st in range(4, NSTILE):
            load_x_tile(st)
        flush_xevict()
        for ci in range(3):
            qkproj_half(wqT, qT, 0, ci, True, 0)
            qkproj_half(wqT, qT, 0, ci, True, 1)
        # sincos after all x tiles: its 22 staging DMAs would otherwise
        # block the x loads on the sync queue (cos->sin stage-tile WAR)
        load_sincos()
        nc.gpsimd.memset(vsb[:, :, :, 0:D], 1.0)
        for ci in range(3):
            qkproj_half(wkT, kT, 0, ci, False, 0)
            qkproj_half(wkT, kT, 0, ci, False, 1)
        rope(qT, 0)
        rope(kT, 0)
        for st in range(NSTILE):
            vproj_st(0, st)

        # row 1 of each weight feeds proj(1), the attention(0) filler
        load_w_row(wq_ext, wqT, 1, tq=nc.sync)
        load_w_row(wk_ext, wkT, 1, tq=nc.sync)
        load_w_row(wv_ext, wvT, 1, tq=nc.sync)

        def vhead_ap(jsz, jt, h):
            return vsb[:jsz, jt, h, :]

        def two_run_ap(t, rows, ilen):
            """[rows, 2, ilen] AP over a [P, 1024] tile: cols {0:ilen} and
            {512:512+ilen} — skips the unwritten hole when ilen < 512.
            For full-width chunks a flat 2D AP is equivalent and cheaper."""
            s = t[:rows, :]
            if ilen == 512:
                return s
            dims = [list(d) for d in s.ap]
            st = dims[-1][0]
            return bass.AP(tensor=s.tensor, offset=s.offset,
                           ap=[dims[0], [512 * st, 2], [st, ilen]])

        exp_f = mybir.ActivationFunctionType.Exp
        scaling = float(D) ** -0.5
        flush_norm = [lambda: None]

        for pt in range(NOTILE):
            # filler units: projections for pt+1 (for pt=4: only the first
            # 3 v-proj tiles — the rest fill attention(5, ic0) itself),
            # Wp loads during attention(0), out-proj row-tiles for pt=5.
            # just-in-time weight streaming: row pt+2 of Wq/Wk/Wv (feeds
            # proj(pt+2)) and one Wp row per pt — spread so no queue ever
            # sees a burst of weight traffic.
            fills = []
            if pt + 2 < NOTILE:
                for w_ext, wT in ((wq_ext, wqT), (wk_ext, wkT), (wv_ext, wvT)):
                    fills.append(lambda w_ext=w_ext, wT=wT:
                                 load_w_row(w_ext, wT, pt + 2, tq=nc.sync))
            if pt < NOTILE - 1:
                fills.append(lambda pt=pt: load_w_row(wp_ext, wpT, pt, tq=nc.sync))
                if pt == NOTILE - 2:
                    fills.append(lambda: load_w_row(wp_ext, wpT, NOTILE - 1,
                                                    tq=nc.sync))
            if pt + 1 < NOTILE:
                np1 = pt + 1
                for ci3 in range(3):
                    for half in range(2):
                        fills.append(lambda ci3=ci3, half=half, np1=np1:
                                     qkproj_half(wqT, qT, np1, ci3, True, half))
                fills.append(lambda np1=np1: rope_dma(qT, np1))
                for ci3 in range(3):
                    for half in range(2):
                        fills.append(lambda ci3=ci3, half=half, np1=np1:
                                     qkproj_half(wkT, kT, np1, ci3, False, half))
                fills.append(lambda np1=np1: rope_dma(kT, np1))
                fills.append(lambda np1=np1: rope_mul(qT, np1))
                vmax = NSTILE if np1 < NOTILE - 1 else 3
                for st in range(vmax):
                    if st == 1:
                        fills.append(lambda np1=np1: rope_mul(kT, np1))
                    fills.append(lambda st=st, np1=np1: vproj_st(np1, st))
            # (for pt=5 the rest of v-proj(5) is emitted inline in the ic0
            # jt loop below — emission order must stay ahead of the PV
            # reads, since Tile tracks dependencies in trace order.)
            stage_fills = {}
            if pt == NOTILE - 1:
                # it 0-3 need ctxT i cols 0:512 (ready after ic0's
                # normalize); it 4-7 need cols up to 1024 (after ic1).
                stage_fills[1] = [lambda it=it: outproj_it(it) for it in range(4)]
                stage_fills[2] = [lambda it=it: outproj_it(it) for it in range(4, 8)]

            state = [0, 0]  # units emitted, paces done (of 39)

            def pace():
                state[1] += 1
                tgt = min(len(fills), -(-len(fills) * state[1] // 45))
                while state[0] < tgt:
                    fills[state[0]]()
                    state[0] += 1

            for ci, (i0, ilen) in enumerate(IC):
                if pt == NOTILE - 1:
                    # out-proj fills read ctxT; the pending normalize must
                    # be emitted before they are
                    flush_norm[0]()
                if ci in stage_fills:
                    fills.extend(stage_fills[ci])
                pvbox = [None]

                def emit_pv(item, pvbox=pvbox, ilen=ilen, pt=pt, ci=ci):
                    if pvbox[0] is None:
                        pvbox[0] = psum_pv.tile([P, 1024], F32, tag="pv",
                                                name=f"pv_{pt}_{ci}")
                    pv = pvbox[0]
                    pes, pjt, pjsz = item
                    for hh in range(2):
                        nc.tensor.matmul(
                            pv[:, 512 * hh:512 * hh + ilen],
                            vhead_ap(pjsz, pjt, 2 * pt + hh),
                            pes[:pjsz, 512 * hh:512 * hh + ilen],
                            start=(pjt == 0), stop=(pjt == NSTILE - 1))

                pending = []
                for jt in range(NSTILE):
                    j0, jsz = _stile(jt)
                    sc = psum_sc.tile([P, 1024], F32, tag="sc",
                                      name=f"sc_{pt}_{ci}_{jt}")
                    for hh in range(2):
                        hb = 64 * hh
                        nc.tensor.matmul(
                            sc[:jsz, 512 * hh:512 * hh + ilen],
                            kT[hb:hb + 64, pt, j0:j0 + jsz],
                            qT[hb:hb + 64, pt, i0:i0 + ilen],
                            start=True, stop=True)
                    es = es_pool.tile([P, 1024], BF16, tag="es",
                                      name=f"es_{pt}_{ci}_{jt}")
                    nc.scalar.activation(out=two_run_ap(es, jsz, ilen),
                                         in_=two_run_ap(sc, jsz, ilen),
                                         func=exp_f, scale=scaling)
                    if jt == 1:
                        # lazy normalize of the previous chunk: emitted
                        # after this chunk's first exps so it never
                        # head-of-line blocks the DVE FIFO
                        flush_norm[0]()
                    if pt == NOTILE - 1 and ci == 0 and jt + 3 < NSTILE:
                        vproj_st(pt, jt + 3)
                    else:
                        pace()
                    if len(pending) >= 2:
                        emit_pv(pending.pop(0))
                    pending.append((es, jt, jsz))
                for item in pending:
                    pace()
                    emit_pv(item)

                def norm(pv=pvbox[0], ilen=ilen, i0=i0, pt=pt, ci=ci):
                    # denominator is replicated on psum rows 0-63
                    rec = rec_pool.tile([D, 1024], F32, tag="rec",
                                        name=f"rec_{pt}_{ci}")
                    nc.vector.reciprocal_approx_fast(
                        out=two_run_ap(rec, D, ilen),
                        in_=two_run_ap(pv, D, ilen))
                    for hh in range(2):
                        nc.vector.tensor_mul(
                            ctxT[64 * hh:64 * hh + 64, pt, i0:i0 + ilen],
                            pv[64:128, 512 * hh:512 * hh + ilen],
                            rec[0:64, 512 * hh:512 * hh + ilen])

                def mk_flush(fn):
                    def f():
                        flush_norm[0] = lambda: None
                        fn()
                    return f

                flush_norm[0] = mk_flush(norm)
                pace()
                pace()
            while state[0] < len(fills):
                fills[state[0]]()
                state[0] += 1

        # ---------------- output projection tail ----------------
        flush_norm[0]()
        for it in range(8, NSTILE):
            outproj_it(it)


_NC_CACHE = None


def get_nc():
    global _NC_CACHE
    if _NC_CACHE is None:
        nc = bacc.Bacc(None, target_bir_lowering=False, debug=False)
        _NC_CACHE = build_kernel(nc)
    return _NC_CACHE


def kernel(**inputs):
    from concourse.bass_utils import run_bass_kernel_spmd

    nc = get_nc()
    names = ["hidden_states", "sin", "cos", "Wq", "bq", "Wk", "Wv", "bv", "Wp", "bp"]
    arrs = {k: np.ascontiguousarray(np.asarray(inputs[k], dtype=np.float32))
            for k in names}
    in_maps = []
    for b in range(B):
        m = {k: arrs[k] for k in names if k != "hidden_states"}
        m["hidden_states"] = np.ascontiguousarray(arrs["hidden_states"][b])
        in_maps.append(m)
    res = run_bass_kernel_spmd(nc, in_maps, core_ids=list(range(B)))
    out = np.stack([res.results[b]["out"] for b in range(B)], axis=0)
    return out.astype(np.float32)


if __name__ == "__main__":
    nc = get_nc()
    print("built ok")
